# revision 6
# baseline (speedup 1.0000x reference)
"""Trainium2 Bass kernel for nn_AdaptiveCapsule (capsule routing).

Strategy (8 NeuronCores, shard in_caps I=1024 -> 128 per core):
  u_hat[b,i,o,d] = sum_e W[i,o,d,e] * x[b,i,e]   (the 34 GFLOP einsum)
  3 routing iterations over out_caps with tiny (64x512 f32) AllReduces.

Device pipeline per core:
  - W and x are passed as uint16 views of the f32 arrays (full f32 byte
    stream goes through HBM; low half-words zeroed so the bf16-interpreted
    lo-partitions are exact zeros, never Inf/NaN).
  - DMA X-bar transpose (2-byte path) streams W_i / x_i from HBM directly
    into SBUF transposed, so the contraction axis E lands on partitions.
    Result tiles interleave [lo,hi] on partitions: even partitions are 0,
    odd partitions are truncated-bf16 values. Matmul contracts K=128 with
    64 live rows; products on even rows are exactly 0*0=0.
  - TensorE: per in-cap pair, 2 column-tiled (M=64) bf16 matmuls, N=512,
    accumulating 8 K-chunks into one PSUM (128,512) f32 tile.
  - Routing on VectorE/ScalarE with u_hat resident in SBUF (128x64x512 f32),
    partition = batch(64)*2 (i-pair rows), free = (i-pair, o*d).
  - 3x AllReduce (64x512 f32 = 128KB) over all 8 cores.
"""

import sys

sys.path.insert(0, "/opt/trn_rl_repo")

import numpy as np
import ml_dtypes

from concourse import bacc, bass, tile
from concourse import mybir
from concourse.bass_utils import run_bass_kernel_spmd

NCORES = 8
B, I, O, D, E = 64, 1024, 16, 32, 512
IL = I // NCORES  # 128 in_caps per core
OD = O * D  # 512
E2 = 2 * E  # 1024 u16 columns per f32 row
PAIRS = IL // 2  # 64 i-pairs per core
NCHUNK = 8  # K chunks of 128 interleaved partitions (64 live e each)
GRP = 4  # i-pairs per routing group
NGRP = PAIRS // GRP
F32 = mybir.dt.float32
BF16 = mybir.dt.bfloat16
AX = mybir.AxisListType
ALU = mybir.AluOpType
ACTF = mybir.ActivationFunctionType

_CACHE = {}


def _softmax(nc, logits, attn, mx, sm):
    # logits/attn: (128, PAIRS, O) f32; mx/sm: (128, PAIRS) scratch
    nc.vector.tensor_reduce(mx[:], logits[:], axis=AX.X, op=ALU.max)
    nc.vector.tensor_tensor(
        attn[:], logits[:], mx[:].unsqueeze(-1).broadcast_to((128, PAIRS, O)),
        op=ALU.subtract,
    )
    nc.scalar.activation(attn[:], attn[:], ACTF.Exp)
    nc.vector.tensor_reduce(sm[:], attn[:], axis=AX.X, op=ALU.add)
    nc.vector.reciprocal(sm[:], sm[:])
    nc.vector.tensor_tensor(
        attn[:], attn[:], sm[:].unsqueeze(-1).broadcast_to((128, PAIRS, O)),
        op=ALU.mult,
    )


def _squash(nc, s, v, sq, n2, nrm, den):
    # s,v: (64, OD); sq: (64, OD); n2/nrm/den: (64, O)
    # squash(s) = norm/(1+norm^2) * s along d
    nc.vector.tensor_tensor(sq[:], s[:], s[:], op=ALU.mult)
    nc.vector.tensor_reduce(
        n2[:], sq[:].rearrange("p (o d) -> p o d", o=O), axis=AX.X, op=ALU.add
    )
    nc.scalar.activation(nrm[:], n2[:], ACTF.Sqrt)
    nc.vector.tensor_scalar_add(den[:], n2[:], 1.0)
    nc.vector.reciprocal(den[:], den[:])
    nc.vector.tensor_tensor(nrm[:], nrm[:], den[:], op=ALU.mult)  # factor
    nc.vector.tensor_tensor(
        v[:].rearrange("p (o d) -> p o d", o=O),
        s[:].rearrange("p (o d) -> p o d", o=O),
        nrm[:].unsqueeze(-1).broadcast_to((64, O, D)),
        op=ALU.mult,
    )


def _build():
    nc = bacc.Bacc(None, target_bir_lowering=False, num_devices=NCORES)
    xu = nc.dram_tensor("xu", [B, IL, E2], BF16, kind="ExternalInput")
    wu = nc.dram_tensor("wu", [IL, OD, E2], BF16, kind="ExternalInput")
    out = nc.dram_tensor("out", [B, OD], F32, kind="ExternalOutput")
    rg = [list(range(NCORES))]

    with tile.TileContext(nc) as tc:
        with (
            tc.tile_pool(name="persist", bufs=1) as persist,
            tc.tile_pool(name="wt", bufs=3) as wpool,
            tc.tile_pool(name="xt", bufs=3) as xpool,
            tc.tile_pool(name="work", bufs=1) as work,
            tc.tile_pool(name="psum", bufs=4, space="PSUM") as psum,
            tc.tile_pool(name="dram", bufs=6, space="DRAM") as dram,
        ):
            uhat = persist.tile([128, PAIRS, OD], F32)

            # ---- Phase 1: u_hat via DMA-transposed bf16 matmuls ----
            for p in range(PAIRS):
                i0 = 2 * p
                xt = xpool.tile([128, 2 * NCHUNK, B], BF16, tag="xt")
                nc.sync.dma_start(out=xt[:], in_=xu[:, i0 : i0 + 2, :], transpose=True)
                ps = psum.tile([128, OD], F32, tag="ps")
                wt0 = wpool.tile([128, NCHUNK, OD], BF16, tag="wt")
                nc.sync.dma_start(out=wt0[:], in_=wu[i0], transpose=True)
                wt1 = wpool.tile([128, NCHUNK, OD], BF16, tag="wt")
                nc.sync.dma_start(out=wt1[:], in_=wu[i0 + 1], transpose=True)
                for c in range(NCHUNK):
                    nc.tensor.matmul(
                        ps[0:64, :], xt[:, c, :], wt0[:, c, :],
                        start=(c == 0), stop=(c == NCHUNK - 1),
                        tile_position=(0, 0),
                    )
                for c in range(NCHUNK):
                    nc.tensor.matmul(
                        ps[64:128, :], xt[:, NCHUNK + c, :], wt1[:, c, :],
                        start=(c == 0), stop=(c == NCHUNK - 1),
                        tile_position=(0, 64),
                    )
                nc.vector.tensor_copy(uhat[:, p, :], ps[:])

            # ---- Routing ----
            logits = persist.tile([128, PAIRS, O], F32)
            attn = persist.tile([128, PAIRS, O], F32)
            mx = persist.tile([128, PAIRS], F32)
            sm = persist.tile([128, PAIRS], F32)
            acc = persist.tile([128, OD], F32)
            s_sb = persist.tile([64, OD], F32)
            v_sb = persist.tile([64, OD], F32)
            v_rep = persist.tile([128, OD], F32)
            sq = persist.tile([64, OD], F32)
            n2 = persist.tile([64, O], F32)
            nrm = persist.tile([64, O], F32)
            den = persist.tile([64, O], F32)
            tmp = work.tile([128, GRP, OD], F32)
            red = work.tile([128, GRP * O], F32)

            for it in range(3):
                if it == 0:
                    # attn uniform = 1/O: s = (1/O) * sum_i u_hat
                    nc.vector.tensor_reduce(
                        acc[:], uhat[:].transpose([0, 2, 1]), axis=AX.X, op=ALU.add
                    )
                else:
                    # logits += sum_d u_hat * v  (uses v of previous iter)
                    for g in range(NGRP):
                        gsl = slice(g * GRP, (g + 1) * GRP)
                        nc.vector.tensor_tensor(
                            tmp[:].rearrange("p g (o d) -> p g o d", o=O),
                            uhat[:, gsl, :].rearrange("p g (o d) -> p g o d", o=O),
                            v_rep[:].rearrange("p (o d) -> p o d", o=O)
                            .unsqueeze(1).broadcast_to((128, GRP, O, D)),
                            op=ALU.mult,
                        )
                        nc.vector.tensor_reduce(
                            red[:],
                            tmp[:].rearrange("p g (o d) -> p (g o) d", o=O),
                            axis=AX.X,
                            op=ALU.add,
                        )
                        rview = red[:].rearrange("p (g o) -> p g o", o=O)
                        if it == 1:
                            nc.vector.tensor_copy(logits[:, gsl, :], rview)
                        else:
                            nc.vector.tensor_tensor(
                                logits[:, gsl, :], logits[:, gsl, :], rview,
                                op=ALU.add,
                            )
                    _softmax(nc, logits, attn, mx, sm)
                    # s = sum_i attn * u_hat
                    for g in range(NGRP):
                        gsl = slice(g * GRP, (g + 1) * GRP)
                        nc.vector.tensor_tensor(
                            tmp[:].rearrange("p g (o d) -> p g o d", o=O),
                            uhat[:, gsl, :].rearrange("p g (o d) -> p g o d", o=O),
                            attn[:, gsl, :].unsqueeze(-1)
                            .broadcast_to((128, GRP, O, D)),
                            op=ALU.mult,
                        )
                        if g == 0:
                            nc.vector.tensor_reduce(
                                acc[:],
                                tmp[:].rearrange("p g (o d) -> p (o d) g", o=O),
                                axis=AX.X,
                                op=ALU.add,
                            )
                        else:
                            red2 = work.tile([128, OD], F32, tag="red2")
                            nc.vector.tensor_reduce(
                                red2[:],
                                tmp[:].rearrange("p g (o d) -> p (o d) g", o=O),
                                axis=AX.X,
                                op=ALU.add,
                            )
                            nc.vector.tensor_tensor(
                                acc[:], acc[:], red2[:], op=ALU.add
                            )

                # fold the two partition halves via DMA-accumulate, then
                # AllReduce s over the 8 cores
                cin = dram.tile([64, OD], F32, tag="cin")
                cout = dram.tile([64, OD], F32, tag="cout")
                nc.sync.dma_start(cin[:], acc[0:64, :])
                nc.gpsimd.dma_start(cin[:], acc[64:128, :], accum_op=ALU.add)
                nc.gpsimd.collective_compute(
                    "AllReduce", ALU.add, replica_groups=rg,
                    ins=[cin[:].opt()], outs=[cout[:].opt()],
                )
                nc.sync.dma_start(s_sb[:], cout[:])
                if it == 0:
                    nc.vector.tensor_scalar_mul(s_sb[:], s_sb[:], 1.0 / O)

                _squash(nc, s_sb, v_sb, sq, n2, nrm, den)
                if it < 2:
                    nc.sync.dma_start(v_rep[0:64, :], v_sb[:])
                    nc.sync.dma_start(v_rep[64:128, :], v_sb[:])

            nc.sync.dma_start(out[:], v_sb[:])

    nc.compile()
    return nc


def _get_nc():
    if "nc" not in _CACHE:
        _CACHE["nc"] = _build()
    return _CACHE["nc"]


def _prep_inputs(x, W, route_bias):
    x = np.asarray(x, dtype=np.float32)
    W = np.asarray(W, dtype=np.float32)
    rb = np.asarray(route_bias, dtype=np.float32)
    if np.any(rb):
        W = W + rb  # reference adds the (1,1,O,1,1) bias onto W
    W0 = np.ascontiguousarray(W.reshape(I, OD, E))
    Wu = W0.view("<u2").copy()
    Wu[..., 0::2] = 0  # zero the f32 low half-words (see module docstring)
    xc = np.ascontiguousarray(x).view("<u2").copy()
    xc[..., 0::2] = 0
    bf = ml_dtypes.bfloat16
    in_maps = []
    for r in range(NCORES):
        sl = slice(r * IL, (r + 1) * IL)
        in_maps.append(
            {
                "wu": np.ascontiguousarray(Wu[sl]).view(bf),
                "xu": np.ascontiguousarray(xc[:, sl, :]).view(bf),
            }
        )
    return in_maps


def kernel(x, W, route_bias, _trace=False, _trace_kwargs=None):
    in_maps = _prep_inputs(x, W, route_bias)
    res = run_bass_kernel_spmd(
        _get_nc(), in_maps, core_ids=list(range(NCORES)),
        trace=_trace, **(_trace_kwargs or {}),
    )
    _CACHE["last_results"] = res
    return np.asarray(res.results[0]["out"], dtype=np.float32).reshape(B, O, D)
